# revision 1
# baseline (speedup 1.0000x reference)
"""Trainium2 Bass kernel for LayerNorm + MLP(16->64->16, ReLU) + residual.

Full inputs in, full output out. Internally: pure data-parallel over 8
NeuronCores (each core gets 16 of the 128 batch rows = 131072 tokens).

Per-core pipeline (token-major chunks of CH tokens, [128, TC, 16] fp32):
  1. reduce-based LN stats: S1 = sum(x), S2 = sum(x^2) per token
     rstd = 16/sqrt(16*S2 - S1^2 + 256*eps)
  2. y = rstd*x - mu*rstd  (bf16, in-place two-op chain)   [DVE + GPSIMD]
  3. DMA xbar transpose (one instr/chunk) -> yT feature-major bf16:
     yT[q, u, c]: partition q = 16*t8 + h, col c = source partition;
     32-partition strips = "pairs" of token sets at 32-aligned bases
  4. mm1: block-diag-2 W1g^T [32,128] @ yT strip -> h psum   [PE, bf16,
     tile_position row strips]; relu + b1 in PSUM->SBUF copy [ACT]
  5. mm2: block-diag-2 W2^T [128,32] @ h, all 4 strips packed into one
     [128,N] psum via tile_position col strips; + b2 in copy  [DVE]
  6. DMA xbar transpose back -> token-major bf16; residual add [DVE]
  7. DMA out fp32

gamma/beta are folded into W1/b1 on the host (exact algebra):
  h = relu(W1*diag(gamma) @ xn + (b1 + W1@beta)),  xn = (x-mu)*rstd
"""

import sys

sys.path.insert(0, "/opt/trn_rl_repo")

import numpy as np
import ml_dtypes
from contextlib import ExitStack

import concourse.bass as bass
import concourse.bacc as bacc
import concourse.tile as tile
from concourse import mybir
from concourse.bass import ds

F32 = mybir.dt.float32
BF16 = mybir.dt.bfloat16
AF = mybir.ActivationFunctionType
OP = mybir.AluOpType

N_CORES = 8
H = 16
D = 64
P = 128
EPS = 1e-5

TOK_FULL = 131072  # tokens per core for the real problem
NBLK = 1024        # matmul free-dim block (bf16 moving operand max)


def _bc(ap2, n):
    """[P, T, 1] AP -> [P, T, n] broadcast AP (inner step 0)."""
    return bass.AP(tensor=ap2.tensor, offset=ap2.offset, ap=[*ap2.ap[:2], [0, n]])


def build_nc(tok=TOK_FULL, ch=16384, debug=False, relu_dve_mod=0, vcopy_act=True,
             bufs_big=8, bufs_small=6, sq_pool=False, resid_pool_mod=0, repeat=1, yadd_pool=False,
             relu_dve_tail=1, chunks=None, relu_head_alt=0, relu_tail_alt=0, d_half=False, smalls_half=False, vcopy_dve_tail=0, ppb_bufs=2, ppc_bufs=2):
    """Trace the single-core bass kernel (SPMD across cores).

    relu_dve_mod: every k-th relu block goes to DVE instead of ACT
    (engine balancing); 0 = all on ACT.
    vcopy_act: run the vT (+b2) PSUM->SBUF copy on ACT instead of DVE.
    """
    if chunks is None:
        assert tok % ch == 0
        chunks = [ch] * (tok // ch)
    assert sum(chunks) == tok
    offs = [sum(chunks[:i]) for i in range(len(chunks))]
    nchunk = len(chunks)
    for c_ in chunks:
        assert c_ % P == 0 and (c_ // P) % 32 == 0

    nc = bacc.Bacc(None, target_bir_lowering=False, debug=debug)
    x_d = nc.dram_tensor("x", [tok, H], F32, kind="ExternalInput")
    o_d = nc.dram_tensor("out", [tok, H], F32, kind="ExternalOutput")
    w1_d = nc.dram_tensor("w1r", [P, P], BF16, kind="ExternalInput")
    w2_d = nc.dram_tensor("w2bd", [P, 32], BF16, kind="ExternalInput")
    b1_d = nc.dram_tensor("b1s", [P, 1], F32, kind="ExternalInput")
    b2_d = nc.dram_tensor("b2s", [P, 1], F32, kind="ExternalInput")

    def chunk_view(d, c):
        tc = chunks[c] // P
        return d[offs[c] : offs[c] + chunks[c], :].rearrange(
            "(p t) h -> p t h", p=P, t=tc
        )

    with tile.TileContext(nc) as tc, ExitStack() as ctx:
        # bufs sized to each tile's pipeline lifetime (stage A -> D skew)
        consts = ctx.enter_context(tc.tile_pool(name="consts", bufs=1))
        px = ctx.enter_context(tc.tile_pool(name="px", bufs=bufs_big))   # A->D
        psq = ctx.enter_context(tc.tile_pool(name="psq", bufs=2))        # A only
        psm = ctx.enter_context(tc.tile_pool(name="psm", bufs=bufs_small))
        py = ctx.enter_context(tc.tile_pool(name="py", bufs=3))          # A only
        pyt = ctx.enter_context(tc.tile_pool(name="pyt", bufs=bufs_small))  # A->C
        ph = ctx.enter_context(tc.tile_pool(name="ph", bufs=3))          # C only
        pvt = ctx.enter_context(tc.tile_pool(name="pvt", bufs=bufs_small))  # C->D
        pvtt = ctx.enter_context(tc.tile_pool(name="pvtt", bufs=3))      # D only
        po = ctx.enter_context(tc.tile_pool(name="po", bufs=3))          # D only
        ppB = ctx.enter_context(tc.tile_pool(name="ppB", bufs=ppb_bufs, space="PSUM"))
        ppC = ctx.enter_context(tc.tile_pool(name="ppC", bufs=ppc_bufs, space="PSUM"))

        w1r = consts.tile([P, P], BF16)  # W1 block-diag replicated 4 strips
        nc.sync.dma_start(w1r[:, :], w1_d[:, :])
        w2s = consts.tile([P, 32], BF16)
        nc.sync.dma_start(w2s[:, :], w2_d[:, :])
        b1s = consts.tile([P, 1], F32)
        nc.sync.dma_start(b1s[:, :], b1_d[:, :])
        b2s = consts.tile([P, 1], F32)
        nc.sync.dma_start(b2s[:, :], b2_d[:, :])
        eps256 = consts.tile([P, 1], F32)
        nc.vector.memset(eps256[:, :], 256.0 * EPS)
        zerob = consts.tile([P, 1], F32)
        nc.vector.memset(zerob[:, :], 0.0)
        warm = consts.tile([P, 1], F32)
        nc.scalar.activation(warm, eps256, AF.Sqrt, bias=zerob[:, :], scale=1.0)
        nc.scalar.activation(warm, eps256, AF.Relu, bias=zerob[:, :], scale=1.0)

        def stage_a(c):
            # load + LN stats + normalize + transpose to feature-major.
            # The load/sq/reduce head of the chain runs at half-chunk
            # granularity to shorten the per-chunk critical path.
            TC = chunks[c] // P
            NSLAB = TC // 8
            HT = TC // 2
            x_vc = chunk_view(x_d, c)
            xt = px.tile([P, TC, H], F32, name="xt")
            sq = psq.tile([P, TC, H], F32, name="sq")
            S1 = psm.tile([P, TC, 1], F32, name="S1")
            S2 = psm.tile([P, TC, 1], F32, name="S2")
            for hh in range(2):
                hsl = slice(HT * hh, HT * hh + HT)
                nc.sync.dma_start(xt[:, hsl, :], x_vc[:, hsl, :])
                if sq_pool:
                    nc.gpsimd.tensor_tensor(
                        sq[:, hsl, :], xt[:, hsl, :], xt[:, hsl, :], op=OP.mult
                    )
                else:
                    nc.scalar.activation(
                        sq[:, hsl, :], xt[:, hsl, :], AF.Square,
                        bias=zerob[:, :], scale=1.0,
                    )
                nc.vector.reduce_sum(
                    S1[:, hsl, :], xt[:, hsl, :], axis=mybir.AxisListType.X
                )
                nc.vector.reduce_sum(
                    S2[:, hsl, :], sq[:, hsl, :], axis=mybir.AxisListType.X
                )
            Gv = psm.tile([P, TC, 1], F32, name="Gv")   # 16*S2 - S1^2 = 256*var
            Psq = psm.tile([P, TC, 1], F32, name="Psq")  # S1^2
            SQv = psm.tile([P, TC, 1], F32, name="SQv")  # 16*sqrt(var+eps)
            R = psm.tile([P, TC, 1], F32, name="R")    # 1/(16*sqrt(var+eps))
            nb = psm.tile([P, TC, 1], F32, name="nb")   # -mu*rstd = -S1*R
            # smalls at half granularity shorten the first-chunk fill chain
            # (y of half 0 no longer waits on half 1's load/stats)
            ssl = [slice(0, HT), slice(HT, TC)] if smalls_half else [slice(0, TC)]
            for sl in ssl:
                nc.vector.tensor_tensor(Psq[:, sl, :], S1[:, sl, :], S1[:, sl, :], op=OP.mult)
                nc.vector.scalar_tensor_tensor(
                    Gv[:, sl, :], S2[:, sl, :], 16.0, Psq[:, sl, :], op0=OP.mult, op1=OP.subtract
                )
                nc.scalar.activation(SQv[:, sl, :], Gv[:, sl, :], AF.Sqrt, bias=eps256[:, :], scale=1.0)
                nc.vector.reciprocal(R[:, sl, :], SQv[:, sl, :])
                nc.vector.scalar_tensor_tensor(nb[:, sl, :], S1[:, sl, :], -1.0, R[:, sl, :], op0=OP.mult, op1=OP.mult)

            # y = (16*R)*x + nb = rstd*x - mu*rstd, then transpose —
            # both at half-chunk granularity so mm1 can start on the first
            # half while the second is still normalizing. The halves write
            # disjoint slab ranges (u) of the same yT tile.
            y = py.tile([P, TC, H], BF16, name="y")
            yT = pyt.tile([P, NSLAB, P], BF16, name="yT")
            for hh in range(2):
                hsl = slice(HT * hh, HT * hh + HT)
                nc.vector.scalar_tensor_tensor(
                    y[:, hsl, :], xt[:, hsl, :], 16.0,
                    _bc(R[:, hsl, :], H), op0=OP.mult, op1=OP.mult
                )
                (nc.gpsimd if yadd_pool else nc.vector).tensor_tensor(
                    y[:, hsl, :], y[:, hsl, :], _bc(nb[:, hsl, :], H), op=OP.add
                )
                usl = slice((NSLAB // 2) * hh, (NSLAB // 2) * (hh + 1))
                nc.sync.dma_start_transpose(
                    yT[:, usl, :],
                    y[:, hsl, :].rearrange("p t h -> p (t h)"),
                )
            return xt, yT

        def stage_c(c, yT):
            # mm1 -> relu -> mm2 -> vT
            TC = chunks[c] // P
            NB = (TC * 16) // NBLK
            yTf = yT[:, :, :].rearrange("q u c -> q (u c)")
            vT = pvt.tile([P, TC * 16], BF16, name="vT")
            for b in range(NB):
                vp = ppC.tile([P, NBLK], F32)  # 2 psum banks; each matmul
                for s in range(4):             # writes one 512-col bank half
                    hp = ppB.tile([P, NBLK], F32)
                    for e in range(NBLK // 512):
                        nc.tensor.matmul(
                            hp[:, ds(512 * e, 512)],
                            w1r[32 * s : 32 * s + 32, :],
                            yTf[32 * s : 32 * s + 32, ds(NBLK * b + 512 * e, 512)],
                            start=True,
                            stop=True,
                            tile_position=(32 * s, 0),
                        )
                    hs = ph.tile([P, NBLK], BF16)
                    tail_dve = relu_dve_tail and c >= nchunk - relu_dve_tail
                    alt = (relu_tail_alt and c >= nchunk - relu_tail_alt) or (
                        relu_head_alt and c < relu_head_alt
                    )
                    if (alt and (4 * b + s) % 2 == 0) or tail_dve or (
                        relu_dve_mod and (4 * b + s) % relu_dve_mod == 0
                    ):
                        nc.vector.tensor_scalar(
                            hs, hp, b1s[:, :], 0.0, op0=OP.add, op1=OP.max
                        )
                    else:
                        nc.scalar.activation(
                            hs, hp, AF.Relu, bias=b1s[:, :], scale=1.0
                        )
                    for e in range(NBLK // 512):
                        nc.tensor.matmul(
                            vp[32 * s : 32 * s + 32, ds(512 * e, 512)],
                            w2s[:, :],
                            hs[:, ds(512 * e, 512)],
                            start=True,
                            stop=True,
                            tile_position=(0, 32 * s),
                        )
                if vcopy_act and not (vcopy_dve_tail and c >= nchunk - vcopy_dve_tail):
                    nc.scalar.activation(
                        vT[:, ds(NBLK * b, NBLK)], vp, AF.Identity,
                        bias=b2s[:, :], scale=1.0,
                    )
                else:
                    nc.vector.tensor_scalar_add(
                        vT[:, ds(NBLK * b, NBLK)], vp, b2s[:, :]
                    )
            return vT

        def stage_d(c, xt, vT):
            # DMA xbar transpose back + residual + store; optionally at
            # half-chunk granularity so the first store starts while the
            # second half is still adding.
            TC = chunks[c] // P
            HT = TC // 2
            vtt = pvtt.tile([P, TC, H], BF16, name="vtt")
            ot = po.tile([P, TC, H], F32, name="ot")
            o_vc = chunk_view(o_d, c)
            eng = nc.gpsimd if (resid_pool_mod and c % resid_pool_mod == 0) else nc.vector
            vtt_v = vtt[:, :, :].rearrange("p t h -> p (t h)").rearrange(
                "p (u c) -> p u c", c=P
            )
            if d_half:
                for hh in range(2):
                    hsl = slice(HT * hh, HT * hh + HT)
                    usl = slice((TC // 16) * hh, (TC // 16) * (hh + 1))
                    nc.sync.dma_start_transpose(
                        vtt_v[:, usl, :], vT[:, ds(HT * 16 * hh, HT * 16)]
                    )
                    eng.tensor_tensor(ot[:, hsl, :], xt[:, hsl, :], vtt[:, hsl, :], op=OP.add)
                    nc.sync.dma_start(o_vc[:, hsl, :], ot[:, hsl, :])
            else:
                nc.sync.dma_start_transpose(vtt_v, vT[:, :])
                eng.tensor_tensor(ot, xt, vtt, op=OP.add)
                nc.sync.dma_start(o_vc, ot[:, :, :])

        # software-pipelined emission (skew=2) so each engine's in-order
        # stream interleaves chunks instead of serializing on the slowest
        # per-chunk dependency chain
        live = {}
        for c0 in range((nchunk + 2) * repeat):
            c = c0 % (nchunk + 2)
            if c < nchunk:
                live[c] = stage_a(c)
            if 1 <= c and c - 1 in live:
                xt, yT = live[c - 1]
                live[c - 1] = (xt, stage_c(c - 1, yT))
            if 2 <= c and c - 2 in live:
                xt, vT = live.pop(c - 2)
                stage_d(c - 2, xt, vT)

    return nc


def host_weights(ln_gamma, ln_beta, w1, b1, w2, b2):
    """Fold gamma/beta into W1/b1; build packed block-diag weights."""
    g = np.asarray(ln_gamma, np.float32)
    be = np.asarray(ln_beta, np.float32)
    w1 = np.asarray(w1, np.float32)
    b1 = np.asarray(b1, np.float32)
    w2 = np.asarray(w2, np.float32)
    b2 = np.asarray(b2, np.float32)

    w1gT = (w1 * g[None, :]).T.astype(ml_dtypes.bfloat16)  # [16, 64]
    b1p = (b1 + w1 @ be).astype(np.float32)                # [64]
    w2T = w2.T.astype(ml_dtypes.bfloat16)                  # [64, 16]

    w1bd = np.zeros((32, 128), ml_dtypes.bfloat16)
    w1bd[0:16, 0:64] = w1gT
    w1bd[16:32, 64:128] = w1gT
    w1r = np.tile(w1bd, (4, 1))                            # [128, 128]
    w2bd = np.zeros((128, 32), ml_dtypes.bfloat16)
    w2bd[0:64, 0:16] = w2T
    w2bd[64:128, 16:32] = w2T
    b1s = np.concatenate([b1p, b1p])[:, None].astype(np.float32)   # [128,1]
    b2s = np.tile(b2, 8)[:, None].astype(np.float32)               # [128,1]
    return w1r, w2bd, b1s, b2s


def kernel(x, ln_gamma, ln_beta, w1, b1, w2, b2):
    from concourse.bass_utils import run_bass_kernel_spmd

    x = np.asarray(x, np.float32)
    B, T, Hh = x.shape
    assert (B, T, Hh) == (128, 8192, 16)
    w1r, w2bd, b1s, b2s = host_weights(ln_gamma, ln_beta, w1, b1, w2, b2)

    xs = x.reshape(N_CORES, TOK_FULL, H)
    in_maps = [
        {
            "x": np.ascontiguousarray(xs[c]),
            "w1r": w1r,
            "w2bd": w2bd,
            "b1s": b1s,
            "b2s": b2s,
        }
        for c in range(N_CORES)
    ]
    nc = build_nc()
    nc.compile()
    res = run_bass_kernel_spmd(nc, in_maps, core_ids=list(range(N_CORES)))
    out = np.stack([np.asarray(res.results[c]["out"]) for c in range(N_CORES)])
    return out.reshape(B, T, Hh).astype(np.float32)


if __name__ == "__main__":
    nc = build_nc(tok=16384, ch=16384)
    print("traced ok")



# revision 21
# speedup vs baseline: 1.5165x; 1.5165x over previous
"""Trainium2 Bass kernel for LayerNorm + MLP(16->64->16, ReLU) + residual.

Full inputs in, full output out. Internally: pure data-parallel over 8
NeuronCores (each core gets 16 of the 128 batch rows = 131072 tokens).

Per-core pipeline (token-major chunks of CH tokens, [128, TC, 16] bf16):
  1. LN stats via DVE bn_stats (quarter-chunk [128,32,16] -> [128,32,6]):
     per token (count, mean, M2) of even/odd feature subsets.
     16*var = M2e + M2o + 4*(m_e - m_o)^2;  2*mu = m_e + m_o
     R2 = 1/sqrt(G/4 + 4*eps)  (=2*rstd/...) so y = (2x - 2mu)*R2
  2. y1 = 2x - 2mu  [DVE stt], y = y1*R2  [Pool stt]  (bf16, in-place)
  3. DMA xbar transpose -> yT feature-major bf16
  4. mm1: block-diag-2 W1g^T [32,128] @ yT strip -> h psum [PE bf16];
     relu + b1 in PSUM->SBUF copy [ACT/DVE split via relu_dve_mod]
  5. mm2: block-diag-2 W2^T [128,32] @ h -> [128,N] psum; +b2 copy [ACT]
  6. DMA xbar transpose back; residual add [DVE half + Pool half]
  7. DMA out bf16 (host upcasts to f32)

HBM I/O is bf16 (host casts x down, out up); rel-err budget 2e-2 absorbs it.
gamma/beta are folded into W1/b1 on the host (exact algebra):
  h = relu(W1*diag(gamma) @ xn + (b1 + W1@beta)),  xn = (x-mu)*rstd
"""

import sys

sys.path.insert(0, "/opt/trn_rl_repo")

import numpy as np
import ml_dtypes
from contextlib import ExitStack

import concourse.bass as bass
import concourse.bacc as bacc
import concourse.tile as tile
from concourse import mybir
from concourse.bass import ds

F32 = mybir.dt.float32
BF16 = mybir.dt.bfloat16
AF = mybir.ActivationFunctionType
OP = mybir.AluOpType

N_CORES = 8
H = 16
D = 64
P = 128
EPS = 1e-5

TOK_FULL = 131072  # tokens per core for the real problem
NBLK = 1024        # matmul free-dim block (bf16 moving operand max)


def _bc(ap2, n):
    """[P, T, 1] AP -> [P, T, n] broadcast AP (inner step 0)."""
    return bass.AP(tensor=ap2.tensor, offset=ap2.offset, ap=[*ap2.ap[:2], [0, n]])


def build_nc(tok=TOK_FULL, ch=16384, debug=False, repeat=1, chunks=None,
             relu_dve_mod=3, relu_dve_tail=0, vcopy_act=True,
             y1_pool=True, y2_pool=True, y2_stt=True, resid_split=False,
             bufs_big=8, bufs_small=6, ppb_bufs=2, ppc_bufs=1,
             skew_c=2, skew_d=4):
    """Trace the single-core bass kernel (SPMD across cores).

    relu_dve_mod: every k-th relu block goes to DVE instead of ACT
    (engine balancing); 0 = all on ACT.
    y2_pool/y2_stt: the y*=R2 op on Pool (as stt) vs DVE.
    resid_split: residual add half on DVE, half on Pool.
    """
    if chunks is None:
        assert tok % ch == 0
        chunks = [ch] * (tok // ch)
    assert sum(chunks) == tok
    offs = [sum(chunks[:i]) for i in range(len(chunks))]
    nchunk = len(chunks)
    for c_ in chunks:
        assert c_ % P == 0 and (c_ // P) % 32 == 0

    nc = bacc.Bacc(None, target_bir_lowering=False, debug=debug)
    x_d = nc.dram_tensor("x", [tok, H], BF16, kind="ExternalInput")
    o_d = nc.dram_tensor("out", [tok, H], BF16, kind="ExternalOutput")
    w1_d = nc.dram_tensor("w1r", [P, P], BF16, kind="ExternalInput")
    w2_d = nc.dram_tensor("w2bd", [P, 32], BF16, kind="ExternalInput")
    b1_d = nc.dram_tensor("b1s", [P, 1], F32, kind="ExternalInput")
    b2_d = nc.dram_tensor("b2s", [P, 1], F32, kind="ExternalInput")

    def chunk_view(d, c):
        tc = chunks[c] // P
        return d[offs[c] : offs[c] + chunks[c], :].rearrange(
            "(p t) h -> p t h", p=P, t=tc
        )

    with tile.TileContext(nc) as tc, ExitStack() as ctx:
        consts = ctx.enter_context(tc.tile_pool(name="consts", bufs=1))
        px = ctx.enter_context(tc.tile_pool(name="px", bufs=bufs_big))   # A->D
        pbn = ctx.enter_context(tc.tile_pool(name="pbn", bufs=3))        # A only
        psm = ctx.enter_context(tc.tile_pool(name="psm", bufs=bufs_small))
        py = ctx.enter_context(tc.tile_pool(name="py", bufs=3))          # A only
        pyt = ctx.enter_context(tc.tile_pool(name="pyt", bufs=bufs_small))  # A->C
        ph = ctx.enter_context(tc.tile_pool(name="ph", bufs=3))          # C only
        pvt = ctx.enter_context(tc.tile_pool(name="pvt", bufs=bufs_small))  # C->D
        pvtt = ctx.enter_context(tc.tile_pool(name="pvtt", bufs=3))      # D only
        po = ctx.enter_context(tc.tile_pool(name="po", bufs=3))          # D only
        ppB = ctx.enter_context(tc.tile_pool(name="ppB", bufs=ppb_bufs, space="PSUM"))
        ppC = ctx.enter_context(tc.tile_pool(name="ppC", bufs=ppc_bufs, space="PSUM"))

        w1r = consts.tile([P, P], BF16)  # W1 block-diag replicated 4 strips
        nc.sync.dma_start(w1r[:, :], w1_d[:, :])
        w2s = consts.tile([P, 32], BF16)
        nc.sync.dma_start(w2s[:, :], w2_d[:, :])
        b1s = consts.tile([P, 1], F32)
        nc.sync.dma_start(b1s[:, :], b1_d[:, :])
        b2s = consts.tile([P, 1], F32)
        nc.sync.dma_start(b2s[:, :], b2_d[:, :])
        epsb = consts.tile([P, 1], F32)
        nc.vector.memset(epsb[:, :], EPS)
        zerob = consts.tile([P, 1], F32)
        nc.vector.memset(zerob[:, :], 0.0)
        warm = consts.tile([P, 1], F32)
        nc.scalar.activation(warm, epsb, AF.Sqrt, bias=zerob[:, :], scale=1.0)
        nc.scalar.activation(warm, epsb, AF.Relu, bias=zerob[:, :], scale=1.0)

        def stage_a(c):
            # load + bn_stats + smalls + normalize + transpose to
            # feature-major. Head of the chain at half-chunk granularity to
            # shorten the per-chunk critical path; bn_stats at quarters
            # (hardware free-size cap of 512).
            TC = chunks[c] // P
            NSLAB = TC // 8
            HT = TC // 2
            x_vc = chunk_view(x_d, c)
            xt = px.tile([P, TC, H], BF16, name="xt")
            sq = pbn.tile([P, TC, H], BF16, name="sq")
            # sum-based LN stats in packed bf16 2D tiles: the packed [P, TC]
            # bf16 output qualifies for the 2x DVE reduce mode (a [P,TC,1]
            # f32 output runs 1x). 16-element sums round once at bf16 write.
            S1 = psm.tile([P, TC], BF16, name="S1")    # sum(x)
            S2 = psm.tile([P, TC], BF16, name="S2")    # sum(x^2)
            P2 = psm.tile([P, TC], BF16, name="P2")    # S1^2
            G = psm.tile([P, TC], BF16, name="G")      # 16*S2 - S1^2 = 256*var
            SQv = psm.tile([P, TC], F32, name="SQv")   # sqrt(var + eps)
            R = psm.tile([P, TC], BF16, name="R")      # rstd
            NB = psm.tile([P, TC], BF16, name="NB")    # -mu*rstd
            with nc.allow_low_precision(reason="LN stats: 16-elem sums round once at bf16 write"):
                for hh in range(2):
                    hsl = slice(HT * hh, HT * hh + HT)
                    nc.sync.dma_start(xt[:, hsl, :], x_vc[:, hsl, :])
                    nc.vector.tensor_tensor(
                        sq[:, hsl, :], xt[:, hsl, :], xt[:, hsl, :], op=OP.mult
                    )
                    nc.vector.reduce_sum(
                        S1[:, hsl], xt[:, hsl, :], axis=mybir.AxisListType.X
                    )
                    nc.vector.reduce_sum(
                        S2[:, hsl], sq[:, hsl, :], axis=mybir.AxisListType.X
                    )
            nc.vector.tensor_tensor(P2[:, :], S1[:, :], S1[:, :], op=OP.mult)
            nc.vector.scalar_tensor_tensor(
                G[:, :], S2[:, :], 16.0, P2[:, :], op0=OP.mult, op1=OP.subtract
            )
            # sqrt(G/256 + eps) = sqrt(var + eps), so R = recip = rstd
            nc.scalar.activation(SQv[:, :], G[:, :], AF.Sqrt,
                                 bias=epsb[:, :], scale=1.0 / 256.0)
            with nc.allow_low_precision(reason="rstd rounds once at bf16 write"):
                nc.vector.reciprocal(R[:, :], SQv[:, :])
                nc.vector.scalar_tensor_tensor(
                    NB[:, :], S1[:, :], -0.0625, R[:, :],
                    op0=OP.mult, op1=OP.mult
                )  # -mu*rstd

            # y = x*rstd - mu*rstd -- two plain tensor_tensor ops so both
            # can run on Pool (the HW Pool engine has no tensor_scalar).
            # Half-chunk granularity so mm1 can start on the first half
            # while the second is still normalizing.
            y = py.tile([P, TC, H], BF16, name="y")
            yT = pyt.tile([P, NSLAB, P], BF16, name="yT")
            for hh in range(2):
                hsl = slice(HT * hh, HT * hh + HT)
                (nc.gpsimd if y1_pool else nc.vector).tensor_tensor(
                    y[:, hsl, :], xt[:, hsl, :], _bc(R[:, hsl], H), op=OP.mult
                )
                (nc.gpsimd if y2_pool else nc.vector).tensor_tensor(
                    y[:, hsl, :], y[:, hsl, :], _bc(NB[:, hsl], H), op=OP.add
                )
                usl = slice((NSLAB // 2) * hh, (NSLAB // 2) * (hh + 1))
                nc.sync.dma_start_transpose(
                    yT[:, usl, :],
                    y[:, hsl, :].rearrange("p t h -> p (t h)"),
                )
            return xt, yT

        def stage_c(c, yT):
            # mm1 -> relu -> mm2 -> vT. The NB block-chains are interleaved
            # (step order alternates b) so PE feeds chain b+1's mm1 while
            # chain b's relu runs — hides the cross-engine relu latency
            # behind PE work instead of stalling the 2-buf PSUM rotation.
            TC = chunks[c] // P
            NB = (TC * 16) // NBLK
            yTf = yT[:, :, :].rearrange("q u c -> q (u c)")
            vT = pvt.tile([P, TC * 16], BF16, name="vT")
            vps = [ppC.tile([P, NBLK], F32, name=f"vp{b}") for b in range(NB)]
            for step in range(4 * NB):
                b, s = step % NB, step // NB
                vp = vps[b]
                hp = ppB.tile([P, NBLK], F32)
                for e in range(NBLK // 512):
                    nc.tensor.matmul(
                        hp[:, ds(512 * e, 512)],
                        w1r[32 * s : 32 * s + 32, :],
                        yTf[32 * s : 32 * s + 32, ds(NBLK * b + 512 * e, 512)],
                        start=True,
                        stop=True,
                        tile_position=(32 * s, 0),
                    )
                hs = ph.tile([P, NBLK], BF16)
                tail_dve = relu_dve_tail and c >= nchunk - relu_dve_tail
                if tail_dve or (relu_dve_mod and step % relu_dve_mod == 0):
                    nc.vector.tensor_scalar(
                        hs, hp, b1s[:, :], 0.0, op0=OP.add, op1=OP.max
                    )
                else:
                    nc.scalar.activation(
                        hs, hp, AF.Relu, bias=b1s[:, :], scale=1.0
                    )
                for e in range(NBLK // 512):
                    nc.tensor.matmul(
                        vp[32 * s : 32 * s + 32, ds(512 * e, 512)],
                        w2s[:, :],
                        hs[:, ds(512 * e, 512)],
                        start=True,
                        stop=True,
                        tile_position=(0, 32 * s),
                    )
                if s == 3:
                    if vcopy_act:
                        nc.scalar.activation(
                            vT[:, ds(NBLK * b, NBLK)], vp, AF.Identity,
                            bias=b2s[:, :], scale=1.0,
                        )
                    else:
                        nc.vector.tensor_scalar_add(
                            vT[:, ds(NBLK * b, NBLK)], vp, b2s[:, :]
                        )
            return vT

        def stage_d(c, xt, vT):
            # DMA xbar transpose back + residual + store (bf16 out).
            TC = chunks[c] // P
            HT = TC // 2
            vtt = pvtt.tile([P, TC, H], BF16, name="vtt")
            ot = po.tile([P, TC, H], BF16, name="ot")
            o_vc = chunk_view(o_d, c)
            vtt_v = vtt[:, :, :].rearrange("p t h -> p (t h)").rearrange(
                "p (u c) -> p u c", c=P
            )
            nc.sync.dma_start_transpose(vtt_v, vT[:, :])
            if resid_split:
                for hh in range(2):
                    hsl = slice(HT * hh, HT * hh + HT)
                    eng = nc.vector if hh == 0 else nc.gpsimd
                    eng.tensor_tensor(
                        ot[:, hsl, :], xt[:, hsl, :], vtt[:, hsl, :], op=OP.add
                    )
            else:
                nc.vector.tensor_tensor(ot, xt, vtt, op=OP.add)
            nc.sync.dma_start(o_vc, ot[:, :, :])

        # software-pipelined emission: stage C runs skew_c chunks behind A
        # and D runs skew_d behind, so each engine's in-order stream only
        # sees work whose cross-engine inputs were enabled iterations ago —
        # deep enough that the ~15-hop per-chunk dependency spine stays off
        # the steady-state critical path. Oldest stages first within each
        # iteration.
        live = {}
        for c0 in range((nchunk + skew_d) * repeat):
            c = c0 % (nchunk + skew_d)
            if skew_c <= c and c - skew_c in live:
                xt, yT = live[c - skew_c]
                live[c - skew_c] = (xt, stage_c(c - skew_c, yT))
            if skew_d <= c and c - skew_d in live:
                xt, vT = live.pop(c - skew_d)
                stage_d(c - skew_d, xt, vT)
            if c < nchunk:
                live[c] = stage_a(c)

    return nc


def host_weights(ln_gamma, ln_beta, w1, b1, w2, b2):
    """Fold gamma/beta into W1/b1; build packed block-diag weights."""
    g = np.asarray(ln_gamma, np.float32)
    be = np.asarray(ln_beta, np.float32)
    w1 = np.asarray(w1, np.float32)
    b1 = np.asarray(b1, np.float32)
    w2 = np.asarray(w2, np.float32)
    b2 = np.asarray(b2, np.float32)

    w1gT = (w1 * g[None, :]).T.astype(ml_dtypes.bfloat16)  # [16, 64]
    b1p = (b1 + w1 @ be).astype(np.float32)                # [64]
    w2T = w2.T.astype(ml_dtypes.bfloat16)                  # [64, 16]

    w1bd = np.zeros((32, 128), ml_dtypes.bfloat16)
    w1bd[0:16, 0:64] = w1gT
    w1bd[16:32, 64:128] = w1gT
    w1r = np.tile(w1bd, (4, 1))                            # [128, 128]
    w2bd = np.zeros((128, 32), ml_dtypes.bfloat16)
    w2bd[0:64, 0:16] = w2T
    w2bd[64:128, 16:32] = w2T
    b1s = np.concatenate([b1p, b1p])[:, None].astype(np.float32)   # [128,1]
    b2s = np.tile(b2, 8)[:, None].astype(np.float32)               # [128,1]
    return w1r, w2bd, b1s, b2s


def prep_x(x):
    """Host-side downcast of x to bf16 (halves input DMA)."""
    return np.asarray(x, np.float32).astype(ml_dtypes.bfloat16)


def kernel(x, ln_gamma, ln_beta, w1, b1, w2, b2):
    from concourse.bass_utils import run_bass_kernel_spmd

    x = np.asarray(x, np.float32)
    B, T, Hh = x.shape
    assert (B, T, Hh) == (128, 8192, 16)
    w1r, w2bd, b1s, b2s = host_weights(ln_gamma, ln_beta, w1, b1, w2, b2)

    xs = prep_x(x).reshape(N_CORES, TOK_FULL, H)
    in_maps = [
        {
            "x": np.ascontiguousarray(xs[c]),
            "w1r": w1r,
            "w2bd": w2bd,
            "b1s": b1s,
            "b2s": b2s,
        }
        for c in range(N_CORES)
    ]
    nc = build_nc()
    nc.compile()
    res = run_bass_kernel_spmd(nc, in_maps, core_ids=list(range(N_CORES)))
    out = np.stack([np.asarray(res.results[c]["out"]) for c in range(N_CORES)])
    return out.reshape(B, T, Hh).astype(np.float32)


if __name__ == "__main__":
    nc = build_nc(tok=16384, ch=16384)
    print("traced ok")
